# revision 14
# baseline (speedup 1.0000x reference)
"""Trainium2 Bass kernel for nn_DiffConvAdaptive (B=32, N=1024, C=768, K=3).

Sharding: data-parallel over batch, 8 cores x 4 samples, no collectives.

Per-core pipeline (B_loc=4, N=1024=32x32, C=768), all fp16 matmul inputs,
fp32 PSUM accumulation:

  Phase 1 (kernel generation, per sample):
    - adaptive_avg_pool1d commutes with the p1 linear layer: pool raw x
      with a precomputed (1024, 9) segment matrix S on the PE.
    - t = silu(xp @ p1_w.T + p1_b) (9 rows/sample)
    - kT chunks [128, 9] computed DIRECTLY transposed on the PE:
      stationary = tsil[:, chunk] (9x128), moving = kg_eff.T (9x9), plus a
      ones x kgb_row matmul for the bias.  kg_eff folds the
      "kernels - sigmoid(beta)*mean" correction on the host.

  Phase 2 (dense stream, per sample):
    - x1 = x @ p2_w.T channel-major via transposed DMA loads of x; PE
      matmuls -> PSUM -> ACT evacuates (+bias) into zero-padded 34x34 conv
      workspaces (fp16).
    - depthwise 3x3 conv runs OFF the PE, split across engines per the
      CONV_ENG table:
        V: DVE tensor_scalar multiplies (4x perf mode) into a 9-slab stage
           tile + 4 in-place tree adds (2x mode).
        A: ACT per-partition-scaled copies into the stage + DVE tree adds.
        G: GPSIMD fused scalar_tensor_tensor MAC chain (fp32 ping-pong).
    - proj computed CHANNEL-MAJOR: stationary = proj_w.T chunk, moving =
      conv chunk; PSUM tiles are DMA'd directly to DRAM (f32).  proj bias
      and the final (C, N) -> (N, C) transpose happen on the host.

  PE therefore runs only matmuls (p2/proj/pool/p1/kgen) in long gapless
  streams, which also keeps the PE DVFS ramp at full clock.
"""

import sys

if "/opt/trn_rl_repo" not in sys.path:
    sys.path.insert(0, "/opt/trn_rl_repo")

import numpy as np

import concourse.bass as bass
import concourse.bacc as bacc
import concourse.mybir as mybir
import concourse.tile as tile
from concourse.bass_utils import run_bass_kernel_spmd

N_CORES = 8
B, N, C = 32, 1024, 768
B_LOC = B // N_CORES
KK = 9
NCH = C // 128   # 6 channel chunks
NTC = N // 128   # 8 token chunks

# conv workspace: 34x34 padded image per partition row (stride 34), pixel
# (y, x) at offset 34*(y+1) + (x+1), zero pad ring.  Tap (dy, dx) reads the
# interior via a [128, 32(stride 34), 32(stride 1)] view at offset
# 34*dy + dx -- pad cells supply the zeros.
WS = 1160

HF = mybir.dt.float16
F32 = mybir.dt.float32

# conv engine per (sample, chunk): first char = mult engine (V=DVE
# tensor_scalar 4x, A=ACT scaled copy), second char = engine for the first
# (4096-elem) tree-add level (g=GPSIMD tensor_tensor, .=DVE).  Remaining
# tree levels always run on DVE.  Last sample avoids GPSIMD so the proj(3)
# tail is not gated on its slow queue.
CONV_ENG = [
    ["Ag", "Ag", "Vg", "Vg", "Vg", "Vg"],
    ["Ag", "Ag", "Vg", "Vg", "Vg", "Vg"],
    ["Ag", "Ag", "Vg", "Vg", "V.", "V."],
    ["A.", "A.", "V.", "V.", "V.", "V."],
]

TAPS = [(dy, dx) for dy in range(3) for dx in range(3)]

_CACHE = {}
LAST_RESULTS = None


def _segment_matrix():
    S = np.zeros((N, KK), np.float32)
    for i in range(KK):
        s = (i * N) // KK
        e = -((-(i + 1) * N) // KK)
        S[s:e, i] = 1.0 / (e - s)
    return S


def build_program():
    # Bacc: its lowering legalizes multi-sem waits (splits drains etc.)
    # that walrus rejects when emitted raw from TileContext on bass.Bass
    nc = bacc.Bacc(None)

    x_d = nc.dram_tensor("xhf", [B_LOC, N, C], HF, kind="ExternalInput")
    wp2T_d = nc.dram_tensor("wp2T", [C, C], HF, kind="ExternalInput")
    wp1T_d = nc.dram_tensor("wp1T", [C, C], HF, kind="ExternalInput")
    wprojT_d = nc.dram_tensor("wprojT", [C, C], HF, kind="ExternalInput")
    S_d = nc.dram_tensor("S", [N, KK], HF, kind="ExternalInput")
    kgT_d = nc.dram_tensor("kgT", [KK, KK], HF, kind="ExternalInput")
    p1b_d = nc.dram_tensor("p1b", [1, C], HF, kind="ExternalInput")
    p2bT_d = nc.dram_tensor("p2bT", [C, 1], F32, kind="ExternalInput")
    kgbr_d = nc.dram_tensor("kgbr", [1, KK], HF, kind="ExternalInput")
    ones_d = nc.dram_tensor("ones", [1, 128], HF, kind="ExternalInput")
    eye128_d = nc.dram_tensor("eye128", [128, 128], HF, kind="ExternalInput")
    outT_d = nc.dram_tensor("outT", [B_LOC, C, N], F32, kind="ExternalOutput")

    add = mybir.AluOpType.add
    mult = mybir.AluOpType.mult

    with tile.TileContext(nc) as tc:
        with (
            tc.tile_pool(name="const", bufs=1) as cpool,
            tc.tile_pool(name="ws", bufs=1) as wspool,
            tc.tile_pool(name="io", bufs=2) as iopool,
            tc.tile_pool(name="xt", bufs=3) as xtpool,
            tc.tile_pool(name="stage", bufs=2) as stpool,
            tc.tile_pool(name="co", bufs=2) as copool,
            tc.tile_pool(name="oo", bufs=3) as oopool,
            tc.tile_pool(name="kgen", bufs=2) as kgpool,
            tc.tile_pool(name="xpT", bufs=12) as xppool,
            tc.tile_pool(name="kT", bufs=1) as ktpool,
            tc.tile_pool(name="psA", bufs=3, space="PSUM") as psA,
            tc.tile_pool(name="psB", bufs=3, space="PSUM") as psB,
            tc.tile_pool(name="psS", bufs=2, space="PSUM") as psS,
        ):
            # ---------------- constants (one batched DMA each) ----------
            wp2T = cpool.tile([128, NCH * C], HF, tag="wp2T")
            nc.sync.dma_start(wp2T[:], wp2T_d[:].rearrange("(k p) d -> p k d", p=128))
            wp1T = cpool.tile([128, NCH * C], HF, tag="wp1T")
            nc.sync.dma_start(wp1T[:], wp1T_d[:].rearrange("(k p) d -> p k d", p=128))
            wprojT = cpool.tile([128, NCH * C], HF, tag="wprojT")
            nc.sync.dma_start(wprojT[:], wprojT_d[:].rearrange("(k p) d -> p k d", p=128))
            S_sb = cpool.tile([128, NTC * KK], HF, tag="S")
            nc.sync.dma_start(S_sb[:], S_d[:].rearrange("(t p) j -> p t j", p=128))
            kgT_sb = cpool.tile([KK, KK], HF, tag="kgT")
            nc.sync.dma_start(kgT_sb[:], kgT_d[:])
            p1b_sb = cpool.tile([1, C], HF, tag="p1b")
            nc.sync.dma_start(p1b_sb[:], p1b_d[:])
            p2bT_sb = cpool.tile([128, NCH], F32, tag="p2bT")
            nc.sync.dma_start(p2bT_sb[:], p2bT_d[:].rearrange("(k p) o -> p k o", p=128))
            kgbr_sb = cpool.tile([1, KK], HF, tag="kgbr")
            nc.sync.dma_start(kgbr_sb[:], kgbr_d[:])
            ones_sb = cpool.tile([1, 128], HF, tag="ones")
            nc.sync.dma_start(ones_sb[:], ones_d[:])
            eye128 = cpool.tile([128, 128], HF, tag="eye128")
            nc.sync.dma_start(eye128[:], eye128_d[:])

            def w_slc(w, kc, c0, ncols):
                return w[:, C * kc + c0:C * kc + c0 + ncols]

            # conv workspaces: pad ring zeroed once, interior overwritten.
            # Two parity sets -> 2-sample pipeline depth.
            ws = {}
            for par in range(2):
                for i in range(NCH):
                    a = wspool.tile([128, WS], HF, tag=f"ws{par}_{i}")
                    nc.gpsimd.memset(a[:], 0.0)
                    ws[(par, i)] = a

            # token-major x (pooling input): one 3D DMA per sample
            xn = {}

            def load_xn(b):
                t = iopool.tile([128, NTC * C], HF, tag="xn", name=f"xn{b}")
                nc.sync.dma_start(
                    t[:], x_d[b].rearrange("(t p) c -> p t c", p=128)
                )
                xn[b] = t

            # channel-major x via transposed DMA (p2 moving operand)
            xt = {}

            def load_xT(b):
                t = xtpool.tile([128, NCH * N], HF, tag="xT", name=f"xT{b}")
                for i in range(NCH):
                    nc.sync.dma_start(
                        t[:, N * i:N * (i + 1)],
                        x_d[b, :, 128 * i:128 * (i + 1)],
                        transpose=True,
                    )
                xt[b] = t

            # SP queue order matters: xn(2)/xn(3) reuse ring slots and wait
            # on phase-1 readers, so the unblocked xT loads go first.
            load_xn(0)
            load_xn(1)
            load_xT(0)
            load_xT(1)
            load_xn(2)
            load_xn(3)
            load_xT(2)

            # ---- phase 1: per-sample kernel-generation chains ----
            kT = {}
            for b in range(B_LOC):
                # pooling: pp[h] accumulates S.T @ x over token chunks
                pp = [psS.tile([KK, 384], F32, tag="pss", name=f"pp{b}_{h}")
                      for h in range(2)]
                for t in range(NTC):
                    for h in range(2):
                        nc.tensor.matmul(
                            pp[h][:],
                            S_sb[:, KK * t:KK * (t + 1)],
                            xn[b][:, C * t + 384 * h:C * t + 384 * (h + 1)],
                            start=(t == 0),
                            stop=(t == NTC - 1),
                        )
                xp = kgpool.tile([KK, C], HF, tag="xp")
                for h in range(2):
                    nc.vector.tensor_copy(xp[:, 384 * h:384 * (h + 1)], pp[h][:])

                # xp -> xpT chunks [128, 9] (p1 stationary)
                xpT = []
                for i in range(NCH):
                    tp = psS.tile([128, KK], HF, tag="pss")
                    nc.tensor.transpose(
                        tp[:], xp[:, 128 * i:128 * (i + 1)], eye128[:KK, :KK]
                    )
                    sb = xppool.tile([128, KK], HF, tag="xpT")
                    nc.vector.tensor_copy(sb[:], tp[:])
                    xpT.append(sb)

                # p1 + silu
                tsil = kgpool.tile([KK, C], HF, tag="tsil")
                for h in range(2):
                    tp1 = psS.tile([KK, 384], F32, tag="pss")
                    nc.tensor.matmul(
                        tp1[:], ones_sb[:1, :KK],
                        p1b_sb[:1, 384 * h:384 * (h + 1)],
                        start=True, stop=False,
                    )
                    for i in range(NCH):
                        nc.tensor.matmul(
                            tp1[:], xpT[i][:],
                            w_slc(wp1T, i, 384 * h, 384),
                            start=False, stop=(i == NCH - 1),
                        )
                    # silu(v) = v * sigmoid(v)
                    sg = kgpool.tile([KK, 384], HF, tag="sg")
                    nc.scalar.activation(
                        sg[:], tp1[:], mybir.ActivationFunctionType.Sigmoid,
                    )
                    nc.vector.tensor_tensor(
                        tsil[:, 384 * h:384 * (h + 1)], tp1[:], sg[:], mult,
                    )

                # kT chunks [128, 9] directly (stationary = tsil 9x128)
                for i in range(NCH):
                    ktp = psS.tile([128, KK], F32, tag="pss")
                    nc.tensor.matmul(
                        ktp[:], tsil[:, 128 * i:128 * (i + 1)], kgT_sb[:],
                        start=True, stop=False,
                    )
                    nc.tensor.matmul(
                        ktp[:], ones_sb[:1, :], kgbr_sb[:1, :],
                        start=False, stop=True,
                    )
                    sb = ktpool.tile([128, KK], F32, tag=f"kT{b}_{i}")
                    nc.vector.tensor_copy(sb[:], ktp[:])
                    kT[(b, i)] = sb

            # ---- conv helpers (per chunk, off-PE) ----
            def tap_view(src, j):
                dy, dx = TAPS[j]
                base = 34 * dy + dx
                v = src[:, base:base + 34 * 32]
                return v.rearrange("p (r e) -> p r e", e=34)[:, :, :32]

            def conv_chunk(spec, src, kt, dst):
                st = stpool.tile([128, KK * N], HF, tag="stage")
                for j in range(KK):
                    out_v = st[:, N * j:N * (j + 1)].rearrange(
                        "p (r e) -> p r e", e=32)
                    if spec[0] == "V":
                        nc.vector.tensor_scalar(
                            out_v, tap_view(src, j), kt[:, j:j + 1],
                            None, mult)
                    else:
                        nc.scalar.activation(
                            out_v, tap_view(src, j),
                            mybir.ActivationFunctionType.Identity,
                            scale=kt[:, j:j + 1],
                        )
                l1 = nc.gpsimd if spec[1] == "g" else nc.vector
                l1.tensor_tensor(
                    st[:, 0:4 * N], st[:, 0:4 * N], st[:, 4 * N:8 * N], add)
                nc.vector.tensor_tensor(
                    st[:, 0:2 * N], st[:, 0:2 * N], st[:, 2 * N:4 * N], add)
                nc.vector.tensor_tensor(
                    st[:, 0:N], st[:, 0:N], st[:, N:2 * N], add)
                nc.vector.tensor_tensor(
                    dst[:], st[:, 0:N], st[:, 8 * N:9 * N], add)

            # ---- phase 2: dense PE stream (p2 -> conv -> proj) ----
            convO = {}

            def emit_proj(b):
                for i in range(NCH):
                    oo = oopool.tile([128, N], F32, tag="oo",
                                     name=f"oo{b}_{i}")
                    for h in range(2):
                        po = psB.tile([128, 512], F32, tag="psb",
                                      name=f"po{b}_{i}_{h}")
                        for kc in range(NCH):
                            nc.tensor.matmul(
                                po[:],
                                w_slc(wprojT, kc, 128 * i, 128),
                                convO[(b, kc)][:, 512 * h:512 * (h + 1)],
                                start=(kc == 0),
                                stop=(kc == NCH - 1),
                            )
                        # PSUM cannot source a DMA and GPSIMD cannot read
                        # PSUM; split the staging copies across ACT/DVE.
                        if h == 0:
                            nc.scalar.copy(oo[:, 512 * h:512 * (h + 1)], po[:])
                        else:
                            nc.vector.tensor_copy(
                                oo[:, 512 * h:512 * (h + 1)], po[:])
                    nc.sync.dma_start(
                        outT_d[b, 128 * i:128 * (i + 1), :], oo[:])

            for b in range(B_LOC):
                par = b % 2
                if b == 1:
                    load_xT(3)
                for i in range(NCH):
                    xps = [psA.tile([128, 512], F32, tag="psa",
                                    name=f"xps{b}_{i}_{h}") for h in range(2)]
                    for kc in range(NCH):
                        for h in range(2):
                            nc.tensor.matmul(
                                xps[h][:],
                                w_slc(wp2T, kc, 128 * i, 128),
                                xt[b][:, N * kc + 512 * h:N * kc + 512 * (h + 1)],
                                start=(kc == 0),
                                stop=(kc == NCH - 1),
                            )
                    for h in range(2):
                        # evacuate into padded rows (+bias, ->fp16)
                        rb = 34 * (1 + 16 * h)
                        dst = ws[(par, i)][:, rb:rb + 544]
                        dst = dst.rearrange("p (r e) -> p r e", e=34)[:, :, 1:33]
                        nc.scalar.activation(
                            dst,
                            xps[h][:].rearrange("p (r e) -> p r e", e=32),
                            mybir.ActivationFunctionType.Identity,
                            bias=p2bT_sb[:, i:i + 1],
                        )
                    cv = copool.tile([128, N], HF, tag=f"cv{i}",
                                     name=f"cv{b}_{i}")
                    conv_chunk(CONV_ENG[b][i], ws[(par, i)],
                               kT[(b, i)][:], cv)
                    convO[(b, i)] = cv
                if b >= 1:
                    emit_proj(b - 1)
            emit_proj(B_LOC - 1)

    nc.finalize()
    return nc


def _prepare_weights(inputs):
    hf = np.float16
    p1_w = np.asarray(inputs["p1_w"], np.float32)
    p1_b = np.asarray(inputs["p1_b"], np.float32)
    kg_w = np.asarray(inputs["kg_w"], np.float32)
    kg_b = np.asarray(inputs["kg_b"], np.float32)
    p2_w = np.asarray(inputs["p2_w"], np.float32)
    p2_b = np.asarray(inputs["p2_b"], np.float32)
    proj_w = np.asarray(inputs["proj_w"], np.float32)
    beta = np.asarray(inputs["beta"], np.float32)

    factor = 1.0 / (1.0 + np.exp(-beta))
    assert np.allclose(factor, factor[0], atol=1e-6), (
        "non-uniform sigmoid(beta) not supported by the host fold"
    )
    A = np.eye(KK, dtype=np.float32) - float(factor[0]) / KK
    kg_w_eff = (A @ kg_w).astype(np.float32)
    kg_b_eff = (A @ kg_b).astype(np.float32)

    return {
        "wp2T": np.ascontiguousarray(p2_w.T).astype(hf),
        "wp1T": np.ascontiguousarray(p1_w.T).astype(hf),
        "wprojT": np.ascontiguousarray(proj_w.T).astype(hf),
        "S": _segment_matrix().astype(hf),
        "kgT": np.ascontiguousarray(kg_w_eff.T).astype(hf),
        "p1b": p1_b.reshape(1, C).astype(hf),
        "p2bT": np.ascontiguousarray(p2_b.reshape(C, 1)),
        "kgbr": kg_b_eff.reshape(1, KK).astype(hf),
        "ones": np.ones((1, 128), hf),
        "eye128": np.eye(128, dtype=np.float16),
    }


def kernel(**inputs):
    global LAST_RESULTS
    if "nc" not in _CACHE:
        _CACHE["nc"] = build_program()
    nc = _CACHE["nc"]

    x = np.asarray(inputs["x"], np.float32)
    proj_b = np.asarray(inputs["proj_b"], np.float32)
    weights = _prepare_weights(inputs)
    xhf = x.astype(np.float16)

    in_maps = []
    for c in range(N_CORES):
        m = dict(weights)
        m["xhf"] = np.ascontiguousarray(xhf[B_LOC * c:B_LOC * (c + 1)])
        in_maps.append(m)

    res = run_bass_kernel_spmd(nc, in_maps, list(range(N_CORES)))
    LAST_RESULTS = res
    outT = np.concatenate(
        [res.results[c]["outT"] for c in range(N_CORES)], axis=0
    )  # (B, C, N) f32
    out = outT.transpose(0, 2, 1) + proj_b
    return np.ascontiguousarray(out.astype(np.float32))
